# revision 14
# baseline (speedup 1.0000x reference)
"""Trainium2 Bass kernel for nn_CoreDiffusion (gnn_message_passing).

Sharding: node dim N=4096 split across 8 cores (512 nodes each).

Host-side staging (per core k):
  - adj is quantized to fp8 e3m4 after centering to [-0.5, 0.5) with
    error-feedback across the c (core-diffusion) dim: q_c = Q(A_c - 0.5 + carry),
    carry' = (A_c - 0.5 + carry) - q_c.  The cumsum over c then telescopes so
    quantization error does not accumulate across GRU steps.  The removed 0.5
    mean is restored on device as a rank-1 bias 0.5*(c+1)*colsum(x16)[d]
    (shipped as the tiny `sbias` input — standard zero-point correction).
  - adj rows for core k are pre-transposed to [B, C, 128(p), JC, NS] so the
    contraction dim j = jc*128+p lands on SBUF partitions with 4KB contiguous
    DMA runs; no on-chip transposes are needed.
  - x is pre-cast to fp16 and packed to the SBUF layout [128, JC, B, D].

Device (per core), software-pipelined over (c, b):
  - Phase A: msgT[d,i] accumulates adj-chunks (fp8e3 moving) against x16
    (fp16 stationary) directly into a per-b persistent PSUM bank, so the
    cumsum over c happens inside PSUM accumulation for free.
  - hx_c = relu(psum + sbias) fused on DVE (tensor_scalar add+max).
  - GRU step (one slot behind Phase A so PE never stalls), processed in two
    NS/2 halves so each gate PSUM tile is a single bank (enables double
    buffering): r|z and i_n|h_n each in one [64,2,NS/2] PSUM tile -> one
    sigmoid per half; gate biases ride the Act activations; pointwise on
    DVE, osum accumulation on Pool.
  - LayerNorm + store, emitted per batch half to overlap the tail.
No collectives; full output gathered on host.
"""
import numpy as np
import ml_dtypes
from contextlib import ExitStack

import concourse.bass as bass
import concourse.mybir as mybir
import concourse.tile as tile
from concourse import bacc
from concourse.masks import make_identity
from concourse.bass_utils import run_bass_kernel_spmd

F32 = mybir.dt.float32
F32R = mybir.dt.float32r
F16 = mybir.dt.float16
F8E3 = mybir.dt.float8e3
AF = mybir.ActivationFunctionType
ALU = mybir.AluOpType

B, C, N, D, H = 2, 4, 4096, 64, 64
NCORES = 8
NS = N // NCORES            # 512 nodes per core
HNS = NS // 2               # GRU half-tile
JC = N // 128               # 32 contraction chunks
NJ = 4                      # adj DMA chunks per (b, c)
JCD = JC // NJ              # 8 contraction chunks per DMA
NXQ = 4                     # x16 DMA quarters
LN_EPS = 1e-5


def build():
    nc = bacc.Bacc("TRN2", target_bir_lowering=False, debug=False,
                   num_devices=NCORES)
    adj_t = nc.declare_dram_parameter("adj_t", [B, C, 128, JC, NS], F8E3,
                                      isOutput=False)
    x_p = nc.declare_dram_parameter("x_p", [128, JC, B, D], F16, isOutput=False)
    sb_in = nc.declare_dram_parameter("sb_in", [64, B, C], F32, isOutput=False)
    w_ih = nc.declare_dram_parameter("w_ih", [3 * H, D], F32, isOutput=False)
    w_hh = nc.declare_dram_parameter("w_hh", [3 * H, H], F32, isOutput=False)
    b_ih = nc.declare_dram_parameter("b_ih", [3 * H], F32, isOutput=False)
    b_hh = nc.declare_dram_parameter("b_hh", [3 * H], F32, isOutput=False)
    gamma = nc.declare_dram_parameter("gamma", [H], F32, isOutput=False)
    beta = nc.declare_dram_parameter("beta", [H], F32, isOutput=False)
    out_s = nc.declare_dram_parameter("out_s", [B, NS, H], F32, isOutput=True)

    with tile.TileContext(nc) as tc, ExitStack() as ctx:
        const = ctx.enter_context(tc.tile_pool(name="const", bufs=1))
        adj_pool = ctx.enter_context(tc.tile_pool(name="adj", bufs=8))
        gru = ctx.enter_context(tc.tile_pool(name="gru", bufs=2))
        psum_acc = ctx.enter_context(tc.tile_pool(name="psA", bufs=1, space="PSUM"))
        psum_g = ctx.enter_context(tc.tile_pool(name="psG", bufs=2, space="PSUM"))
        psum_m = ctx.enter_context(tc.tile_pool(name="psM", bufs=2, space="PSUM"))

        # ---------- setup (cheap, non-blocking) ----------
        ident = const.tile([128, 128], F32)
        make_identity(nc, ident)
        eps_sb = const.tile([128, 1], F32)
        nc.vector.memset(eps_sb, LN_EPS)

        # x16 quarters + first adj chunks interleaved on the sync queue so the
        # first Phase-A matmul can start ~3.5us in.
        x16 = const.tile([128, JC, B, D], F16)
        a_first = []
        for q in range(NXQ):
            qs = JC // NXQ
            nc.sync.dma_start(x16[:, q * qs:(q + 1) * qs, :, :],
                              x_p[:, q * qs:(q + 1) * qs, :, :])
            a_in = adj_pool.tile([128, JCD, NS], F8E3, tag="a_in")
            nc.sync.dma_start(a_in, adj_t[0, 0, :, q * JCD:(q + 1) * JCD, :])
            a_first.append(a_in)
        sbias = const.tile([64, B, C], F32)
        nc.sync.dma_start(sbias, sb_in[:, :, :])
        wih_sb = const.tile([128, 2, D], F32)
        nc.sync.dma_start(wih_sb[:, 0, :], w_ih[0:128, :])
        nc.sync.dma_start(wih_sb[0:64, 1, :], w_ih[128:192, :])
        whh_sb = const.tile([128, 2, H], F32)
        nc.sync.dma_start(whh_sb[:, 0, :], w_hh[0:128, :])
        nc.sync.dma_start(whh_sb[0:64, 1, :], w_hh[128:192, :])

        # gate biases: bsum = b_ih + b_hh (for r, z); b_ih_n / b_hh_n separate
        bsum = const.tile([64, 3], F32)
        bih_sb = const.tile([64, 3], F32)
        nc.sync.dma_start(bih_sb, b_ih.rearrange("(g p) -> p g", p=64))
        bhh_sb = const.tile([64, 3], F32)
        nc.sync.dma_start(bhh_sb, b_hh.rearrange("(g p) -> p g", p=64))
        nc.vector.tensor_add(bsum, bih_sb, bhh_sb)

        gam_sb = const.tile([128, H], F32)
        g_ap = gamma[:]
        nc.gpsimd.dma_start(out=gam_sb, in_=bass.AP(
            tensor=g_ap.tensor, offset=g_ap.offset, ap=[[0, 128]] + list(g_ap.ap)))
        bet_sb = const.tile([128, H], F32)
        b_ap = beta[:]
        nc.gpsimd.dma_start(out=bet_sb, in_=bass.AP(
            tensor=b_ap.tensor, offset=b_ap.offset, ap=[[0, 128]] + list(b_ap.ap)))

        # persistent state
        hx = const.tile([64, C, B * NS], F32R)
        h_t = const.tile([64, B * NS], F32R)
        osum = const.tile([64, B * NS], F32)

        ps_acc = psum_acc.tile([64, B, NS], F32)    # per-b running cumsum
        wg = [const.tile([64, 64], F32R, name=f"wg{i}")
              for i in range(6)]  # rx,zx,nx,rh,zh,nh

        def emit_phase_a(c, b, chunks=None):
            for jd in range(NJ):
                if chunks is not None:
                    a_in = chunks[jd]
                else:
                    a_in = adj_pool.tile([128, JCD, NS], F8E3, tag="a_in")
                    nc.sync.dma_start(
                        a_in, adj_t[b, c, :, jd * JCD:(jd + 1) * JCD, :])
                for jl in range(JCD):
                    jc = jd * JCD + jl
                    nc.tensor.matmul(
                        ps_acc[:, b, :], x16[:, jc, b, :], a_in[:, jl, :],
                        start=(c == 0 and jc == 0), stop=(jc == JC - 1),
                        skip_group_check=True)
            # hx_c = relu(cumsum + 0.5*(c+1)*colsum(x)) fused on DVE
            nc.vector.tensor_scalar(
                out=hx[:, c, b * NS:(b + 1) * NS], in0=ps_acc[:, b, :],
                scalar1=sbias[:, b, c:c + 1], scalar2=0.0,
                op0=ALU.add, op1=ALU.max)

        def emit_gru(c, b):
            for hf in range(2):
                sl = slice(b * NS + hf * HNS, b * NS + (hf + 1) * HNS)
                hx_c = hx[:, c, sl]
                # One start=True per 2KB PSUM zero region: the start marks the
                # whole region pending-zero, so the sibling gate's first write
                # (start=False) still lands on zeroed bytes.
                ps_rz = psum_g.tile([64, 2, HNS], F32, tag="rz")
                nc.tensor.matmul(ps_rz[:, 0, :], wg[0], hx_c,
                                 start=True, stop=False)
                nc.tensor.matmul(ps_rz[:, 1, :], wg[1], hx_c,
                                 start=False, stop=(c == 0))
                ps_nh = psum_g.tile([64, 2, HNS], F32, tag="nh")
                nc.tensor.matmul(ps_nh[:, 0, :], wg[2], hx_c,
                                 start=True, stop=(c == 0))
                if c > 0:
                    nc.tensor.matmul(ps_rz[:, 0, :], wg[3], h_t[:, sl],
                                     start=False, stop=False)
                    nc.tensor.matmul(ps_rz[:, 1, :], wg[4], h_t[:, sl],
                                     start=False, stop=True)
                    nc.tensor.matmul(ps_nh[:, 1, :], wg[5], h_t[:, sl],
                                     start=False, stop=True)
                rz = gru.tile([64, 2, HNS], F32, tag="rz")
                nc.scalar.activation(rz[:, 0, :], ps_rz[:, 0, :], AF.Sigmoid,
                                     bias=bsum[:, 0:1])
                nc.scalar.activation(rz[:, 1, :], ps_rz[:, 1, :], AF.Sigmoid,
                                     bias=bsum[:, 1:2])
                n_sb = gru.tile([64, HNS], F32, tag="n")
                if c > 0:
                    t0 = gru.tile([64, HNS], F32, tag="t0")
                    nc.vector.tensor_scalar_add(t0, ps_nh[:, 1, :],
                                                bhh_sb[:, 2:3])
                    t1 = gru.tile([64, HNS], F32, tag="t1")
                    nc.vector.tensor_mul(t1, rz[:, 0, :], t0)
                    t2 = gru.tile([64, HNS], F32, tag="t2")
                    nc.vector.tensor_add(t2, t1, ps_nh[:, 0, :])
                    nc.scalar.activation(n_sb, t2, AF.Tanh, bias=bih_sb[:, 2:3])
                else:
                    nc.scalar.activation(n_sb, ps_nh[:, 0, :], AF.Tanh,
                                         bias=bih_sb[:, 2:3])
                # h' = n + z*(h - n)   (c=0: h=0 -> h' = n - z*n)
                t3 = gru.tile([64, HNS], F32, tag="t3")
                if c > 0:
                    nc.vector.tensor_sub(t3, h_t[:, sl], n_sb)
                else:
                    nc.vector.tensor_scalar_mul(t3, n_sb, -1.0)
                t4 = gru.tile([64, HNS], F32, tag="t4")
                nc.vector.tensor_mul(t4, rz[:, 1, :], t3)
                nc.vector.tensor_add(h_t[:, sl], n_sb, t4)
                if c == 0:
                    nc.gpsimd.tensor_copy(osum[:, sl], h_t[:, sl])
                else:
                    nc.gpsimd.tensor_add(osum[:, sl], osum[:, sl], h_t[:, sl])

        def emit_ln(b):
            nblk = NS // 128
            base = b * nblk
            oT = const.tile([128, nblk, H], F32, name=f"oT{b}")
            stats = const.tile([128, nblk, 6], F32, name=f"st{b}")
            mv = const.tile([128, nblk, 2], F32, name=f"mv{b}")
            rstd = const.tile([128, nblk, 1], F32, name=f"rs{b}")
            out_st = const.tile([128, nblk, H], F32, name=f"os{b}")
            for blk in range(nblk):
                ps_o = psum_m.tile([128, 64], F32, tag="m")
                nc.tensor.transpose(ps_o, osum[:, bass.ts(base + blk, 128)],
                                    ident[0:64, 0:64])
                nc.vector.tensor_copy(oT[:, blk, :], ps_o)
            for blk in range(nblk):
                nc.vector.bn_stats(stats[:, blk, :], oT[:, blk, :])
                nc.vector.bn_aggr(mv[:, blk, :], stats[:, blk, :])
            for blk in range(nblk):
                nc.scalar.activation(rstd[:, blk, :], mv[:, blk, 1:2],
                                     AF.Sqrt, bias=eps_sb)
            for blk in range(nblk):
                nc.vector.reciprocal(rstd[:, blk, :], rstd[:, blk, :])
                xm = gru.tile([128, H], F32, tag="xm")
                nc.vector.tensor_scalar_sub(xm, oT[:, blk, :], mv[:, blk, 0:1])
                nc.vector.tensor_scalar_mul(xm, xm, rstd[:, blk, :])
                nc.vector.tensor_mul(xm, xm, gam_sb)
                nc.vector.tensor_add(out_st[:, blk, :], xm, bet_sb)
            nc.sync.dma_start(
                out_s[b].rearrange("(q p) d -> p q d", p=128), out_st)

        # ---------- main loop ----------
        emit_phase_a(0, 0, chunks=a_first)
        # GRU weight stationaries: transpose after slot 0's matmuls so they
        # do not delay the first Phase-A work on PE.
        for gi, (src, blk, prow, tag) in enumerate([
                (wih_sb, 0, 0, "rz"), (wih_sb, 0, 64, "nh"), (wih_sb, 1, 0, "m"),
                (whh_sb, 0, 0, "rz"), (whh_sb, 0, 64, "nh"), (whh_sb, 1, 0, "m")]):
            pool_w = psum_m if tag == "m" else psum_g
            ps_w = pool_w.tile([64, 64], F32, tag=tag, name=f"psw{gi}")
            nc.tensor.transpose(ps_w, src[prow:prow + 64, blk, :],
                                ident[prow:prow + 64, prow:prow + 64])
            nc.vector.tensor_copy(wg[gi], ps_w)
        pending = (0, 0)
        for c in range(C):
            for b in range(B):
                if (c, b) == (0, 0):
                    continue
                emit_phase_a(c, b)
                emit_gru(*pending)
                pending = (c, b)
        emit_gru(*pending)
        emit_ln(0)
        emit_ln(1)

    nc.compile()
    return nc


_NC_CACHE = None


def _get_nc():
    global _NC_CACHE
    if _NC_CACHE is None:
        _NC_CACHE = build()
    return _NC_CACHE


def _quantize_adj(adj):
    """Center to [-0.5, 0.5), quantize to fp8 e3m4 with error feedback
    across the c dim so the on-device cumsum telescopes the error."""
    q = np.empty((B, C, N, N), dtype=ml_dtypes.float8_e3m4)
    carry = np.zeros((B, N, N), np.float32)
    for c in range(C):
        t = (adj[:, c] - 0.5) + carry
        qc = t.astype(ml_dtypes.float8_e3m4)
        carry = t - qc.astype(np.float32)
        q[:, c] = qc
    return q


def run(inputs, **spmd_kwargs):
    nc = _get_nc()
    adj = np.asarray(inputs["adj"], dtype=np.float32)
    adj8 = _quantize_adj(adj)
    xq = np.asarray(inputs["x"], dtype=np.float32).astype(np.float16)
    # [128(p), JC, B, D] with j = jc*128 + p
    x_p = np.ascontiguousarray(xq.reshape(B, JC, 128, D).transpose(2, 1, 0, 3))
    # zero-point correction: sbias[d, b, c] = 0.5*(c+1)*sum_j x16[b, j, d]
    colsum = xq.astype(np.float32).sum(axis=1)            # [B, D]
    sb = np.ascontiguousarray(
        np.einsum('bd,c->dbc', colsum, 0.5 * np.arange(1, C + 1,
                                                       dtype=np.float32)))
    in_maps = []
    for k in range(NCORES):
        # [B, C, 128(p), JC, NS] with j = jc*128 + p, i local to core k
        a_k = adj8[:, :, k * NS:(k + 1) * NS, :]              # [B, C, NS(i), N(j)]
        a_k = a_k.reshape(B, C, NS, JC, 128).transpose(0, 1, 4, 3, 2)
        m = {
            "adj_t": np.ascontiguousarray(a_k),
            "x_p": x_p,
            "sb_in": sb,
            "w_ih": np.ascontiguousarray(inputs["w_ih"], dtype=np.float32),
            "w_hh": np.ascontiguousarray(inputs["w_hh"], dtype=np.float32),
            "b_ih": np.ascontiguousarray(inputs["b_ih"], dtype=np.float32),
            "b_hh": np.ascontiguousarray(inputs["b_hh"], dtype=np.float32),
            "gamma": np.ascontiguousarray(inputs["gamma"], dtype=np.float32),
            "beta": np.ascontiguousarray(inputs["beta"], dtype=np.float32),
        }
        in_maps.append(m)
    res = run_bass_kernel_spmd(nc, in_maps, list(range(NCORES)), **spmd_kwargs)
    out = np.concatenate([res.results[k]["out_s"] for k in range(NCORES)], axis=1)
    return out.astype(np.float32), res


def kernel(**inputs):
    out, _ = run(inputs)
    return out
